# revision 6
# baseline (speedup 1.0000x reference)
"""Trainium2 Bass kernel for nn_InteractionHead (HOI pair label assignment).

Reference semantics (per image b):
    P = N*N candidate (human, object) index pairs (i, j).
    valid[p=(i,j)] = (det_labels[i] == 49) and (i != j)
    iou[p, m] = min(IoU(boxes[i], boxes_h[m]), IoU(boxes[j], boxes_o[m]))
    labels[p, c] = valid[p] * min(1, sum_m (iou[p,m] >= 0.5) * (hoi[m] == c))

Key identity: IoU(boxes[i], boxes_h[m]) only depends on (i, m), so the whole
problem reduces to two [M, N] binary match matrices per image:
    Hh[m, i] = IoU(boxes[i], boxes_h[m]) >= 0.5   (masked by det_labels[i]==49)
    Ho[m, j] = IoU(boxes[j], boxes_o[m]) >= 0.5
    labels[(i,j), c] = clip(sum_m Hh[m,i] * Ho[m,j] * onehot[m,c]) * (i != j)

Per core (1 image): precompute ST[m, i, j] = Hh[m,i]*Ho[m,j] (bf16, exact 0/1),
then for each i: matmul ST[:, i, :].T @ onehot -> psum[128 j, 600 c], evacuate
with a clip-to-{0,1} op whose per-partition scale operand zeroes row j == i,
DMA the [128, 600] f32 tile to rows [i*128, (i+1)*128) of the output.

Sharding: data-parallel over batch dim B=8 across the 8 NeuronCores.
"""

import os
import numpy as np

B, N, M, C = 8, 128, 32, 600
HUMAN_ID = 49
P = N * N

_CACHE = {}


def _build_nc():
    import concourse.bass as bass
    import concourse.bacc as bacc
    import concourse.tile as tile_mod
    from concourse import mybir
    from contextlib import ExitStack

    f32 = mybir.dt.float32
    bf16 = mybir.dt.bfloat16
    Alu = mybir.AluOpType

    nc = bacc.Bacc("TRN2")
    t_boxes = nc.dram_tensor("boxes", [N, 4], f32, kind="ExternalInput")
    t_bh = nc.dram_tensor("bh", [M, 4], f32, kind="ExternalInput")
    t_bo = nc.dram_tensor("bo", [M, 4], f32, kind="ExternalInput")
    t_det = nc.dram_tensor("det", [N], f32, kind="ExternalInput")
    t_hoi = nc.dram_tensor("hoi", [M], f32, kind="ExternalInput")
    t_lab = nc.dram_tensor("labels", [P, C], f32, kind="ExternalOutput")
    lab_ap = t_lab.ap()

    with tile_mod.TileContext(nc) as tc:
        with ExitStack() as ctx:
            const = ctx.enter_context(tc.tile_pool(name="const", bufs=1))
            psum = ctx.enter_context(tc.tile_pool(name="psum", bufs=4, space="PSUM"))
            outp = ctx.enter_context(tc.tile_pool(name="outp", bufs=8))

            # ---- load inputs (tiny) ----
            # coord[m, n] = boxes[n, k], broadcast over the M partitions.
            # One tile per coordinate so no consumer waits on >2 DMA sems
            # (walrus has a small per-instruction sync-wait budget).
            coords = []
            for k in range(4):
                ct = const.tile([M, N], f32, tag=f"coord{k}")
                nc.gpsimd.dma_start(
                    out=ct[:, :],
                    in_=bass.AP(tensor=t_boxes, offset=k, ap=[[0, M], [4, N]]),
                )
                coords.append(ct)
            bht = const.tile([M, 4], f32)
            nc.gpsimd.dma_start(out=bht[:, :], in_=t_bh.ap())
            bot = const.tile([M, 4], f32)
            nc.gpsimd.dma_start(out=bot[:, :], in_=t_bo.ap())
            # det broadcast over M partitions: detb[m, n] = det[n]
            detb = const.tile([M, N], f32)
            nc.gpsimd.dma_start(
                out=detb[:, :],
                in_=bass.AP(tensor=t_det, offset=0, ap=[[0, M], [1, N]]),
            )
            hoit = const.tile([M, 1], f32)
            nc.gpsimd.dma_start(
                out=hoit[:, :],
                in_=bass.AP(tensor=t_hoi, offset=0, ap=[[1, M], [1, 1]]),
            )

            X1, Y1, X2, Y2 = coords

            # ---- area of each det box, replicated on all M partitions ----
            dbx = const.tile([M, N], f32)
            nc.vector.tensor_sub(dbx, X2, X1)
            dby = const.tile([M, N], f32)
            nc.vector.tensor_sub(dby, Y2, Y1)
            areaB = const.tile([M, N], f32)
            nc.vector.tensor_mul(areaB, dbx, dby)

            # ---- binary IoU-match matrices vs boxes_h and boxes_o ----
            # match <=> iou >= 0.5 <=> 3*inter >= areaA + areaB  (areas > 0)
            def iou_bin(src, out_dtype, tag):
                Sx1, Sy1, Sx2, Sy2 = (src[:, k : k + 1] for k in range(4))
                lt = const.tile([M, N], f32, tag=f"lt{tag}")
                nc.vector.tensor_scalar_max(lt, X1, Sx1)
                rb = const.tile([M, N], f32, tag=f"rb{tag}")
                nc.vector.tensor_scalar_min(rb, X2, Sx2)
                w = const.tile([M, N], f32, tag=f"w{tag}")
                nc.vector.tensor_sub(w, rb, lt)
                nc.vector.tensor_scalar_max(w, w, 0.0)
                lty = const.tile([M, N], f32, tag=f"lty{tag}")
                nc.vector.tensor_scalar_max(lty, Y1, Sy1)
                rby = const.tile([M, N], f32, tag=f"rby{tag}")
                nc.vector.tensor_scalar_min(rby, Y2, Sy2)
                h = const.tile([M, N], f32, tag=f"h{tag}")
                nc.vector.tensor_sub(h, rby, lty)
                nc.vector.tensor_scalar_max(h, h, 0.0)
                inter = const.tile([M, N], f32, tag=f"inter{tag}")
                nc.vector.tensor_mul(inter, w, h)
                # inter := 3*inter - areaB
                nc.vector.tensor_scalar_mul(inter, inter, 3.0)
                nc.vector.tensor_sub(inter, inter, areaB)
                adx = const.tile([M, 1], f32, tag=f"adx{tag}")
                nc.vector.tensor_sub(adx, Sx2, Sx1)
                ady = const.tile([M, 1], f32, tag=f"ady{tag}")
                nc.vector.tensor_sub(ady, Sy2, Sy1)
                areaS = const.tile([M, 1], f32, tag=f"areaS{tag}")
                nc.vector.tensor_mul(areaS, adx, ady)
                binS = const.tile([M, N], out_dtype, tag=f"bin{tag}")
                nc.vector.tensor_scalar(
                    out=binS, in0=inter, scalar1=areaS, scalar2=None, op0=Alu.is_ge
                )
                return binS

            bin_h = iou_bin(bht, f32, "h")
            bin_o = iou_bin(bot, bf16, "o")

            # fold det_labels == HUMAN_ID into the human-side matrix
            vb = const.tile([M, N], f32)
            nc.vector.tensor_scalar(
                out=vb, in0=detb, scalar1=float(HUMAN_ID), scalar2=None,
                op0=Alu.is_equal,
            )
            hh_m = const.tile([M, N], bf16)
            nc.vector.tensor_mul(hh_m, bin_h, vb)

            # ---- onehot[m, c] = (hoi[m] == c) ----
            iota_c = const.tile([M, C], f32)
            nc.gpsimd.iota(
                iota_c[:, :], pattern=[[1, C]], base=0, channel_multiplier=0,
                allow_small_or_imprecise_dtypes=True,
            )
            onehot = const.tile([M, C], bf16)
            nc.vector.tensor_scalar(
                out=onehot, in0=iota_c, scalar1=hoit[:, 0:1], scalar2=None,
                op0=Alu.is_equal,
            )

            # ---- ST[m, i, j] = hh_m[m, i] * bin_o[m, j] ----
            ST = const.tile([M, N, N], bf16)
            nc.vector.tensor_tensor(
                out=ST[:, :, :],
                in0=bin_o[:, :].unsqueeze(1).broadcast_to([M, N, N]),
                in1=hh_m[:, :].unsqueeze(2).broadcast_to([M, N, N]),
                op=Alu.mult,
            )

            # ---- Sm[j, i] = 0 if i == j else 1  (per-partition scale cols) ----
            iota_d = const.tile([N, N], f32)
            nc.gpsimd.iota(
                iota_d[:, :], pattern=[[-1, N]], base=0, channel_multiplier=1,
                allow_small_or_imprecise_dtypes=True,
            )
            Sm = const.tile([N, N], f32)
            nc.vector.tensor_scalar(
                out=Sm, in0=iota_d, scalar1=0.0, scalar2=None, op0=Alu.not_equal
            )

            # ---- main loop over human index i ----
            Sign = mybir.ActivationFunctionType.Sign
            for i in range(N):
                ps = psum.tile([N, C], f32)
                st_i = ST[:, i, :]
                nc.tensor.matmul(
                    ps[:, 0:512], st_i, onehot[:, 0:512], start=True, stop=True
                )
                nc.tensor.matmul(
                    ps[:, 512:C], st_i, onehot[:, 512:C], start=True, stop=True
                )
                ob = outp.tile([N, C], f32)
                if i % 2 == 0:
                    # Sign(psum * Sm[:, i]): psum in {0..32}; row j==i scaled to 0
                    nc.scalar.activation(
                        out=ob[:, :], in_=ps[:, :], func=Sign, bias=0.0,
                        scale=Sm[:, i : i + 1],
                    )
                else:
                    nc.vector.tensor_scalar(
                        out=ob[:, :], in0=ps[:, :], scalar1=0.5,
                        scalar2=Sm[:, i : i + 1], op0=Alu.is_ge, op1=Alu.mult,
                    )
                nc.sync.dma_start(
                    out=lab_ap[i * N : (i + 1) * N, :], in_=ob[:, :]
                )
    nc.compile()
    return nc


def _get_nc():
    if "nc" not in _CACHE:
        _CACHE["nc"] = _build_nc()
    return _CACHE["nc"]


def kernel(boxes, boxes_h, boxes_o, det_labels, hoi):
    from concourse import bass_utils

    boxes = np.ascontiguousarray(np.asarray(boxes), dtype=np.float32)
    boxes_h = np.ascontiguousarray(np.asarray(boxes_h), dtype=np.float32)
    boxes_o = np.ascontiguousarray(np.asarray(boxes_o), dtype=np.float32)
    det_f = np.asarray(det_labels).astype(np.float32)
    hoi_f = np.asarray(hoi).astype(np.float32)

    nc = _get_nc()
    in_maps = [
        {
            "boxes": boxes[b],
            "bh": boxes_h[b],
            "bo": boxes_o[b],
            "det": np.ascontiguousarray(det_f[b]),
            "hoi": np.ascontiguousarray(hoi_f[b]),
        }
        for b in range(B)
    ]
    trace = bool(int(os.environ.get("KERNEL_TRACE", "0")))
    res = bass_utils.run_bass_kernel_spmd(
        nc, in_maps, core_ids=list(range(B)), trace=trace
    )
    _CACHE["last_results"] = res

    labels = np.stack([res.results[b]["labels"] for b in range(B)], axis=0)
    hi, oi = np.meshgrid(np.arange(N), np.arange(N), indexing="ij")
    paired_idx = np.stack([hi.ravel(), oi.ravel()], axis=-1).astype(np.int32)
    return paired_idx, labels


# revision 7
# speedup vs baseline: 1.1343x; 1.1343x over previous
"""Trainium2 Bass kernel for nn_InteractionHead (HOI pair label assignment).

Reference semantics (per image b):
    P = N*N candidate (human, object) index pairs (i, j).
    valid[p=(i,j)] = (det_labels[i] == 49) and (i != j)
    iou[p, m] = min(IoU(boxes[i], boxes_h[m]), IoU(boxes[j], boxes_o[m]))
    labels[p, c] = valid[p] * min(1, sum_m (iou[p,m] >= 0.5) * (hoi[m] == c))

Key identity: IoU(boxes[i], boxes_h[m]) only depends on (i, m), so the whole
problem reduces to two [M, N] binary match matrices per image:
    Hh[m, i] = IoU(boxes[i], boxes_h[m]) >= 0.5   (masked by det_labels[i]==49)
    Ho[m, j] = IoU(boxes[j], boxes_o[m]) >= 0.5
    labels[(i,j), c] = clip(sum_m Hh[m,i] * Ho[m,j] * onehot[m,c]) * (i != j)

Per core (1 image): precompute ST[m, i, j] = Hh[m,i]*Ho[m,j] (bf16, exact 0/1,
built in 4 chunks so matmuls can start early), then for each i: matmul
ST[:, i, :].T @ onehot -> psum[128 j, 600 c], evacuate with a clip-to-{0,1} op
whose per-partition scale operand zeroes row j == i, DMA the [128, 600] f32
tile to rows [i*128, (i+1)*128) of the output. Output stores alternate between
the two HWDGE rings (SP / ACT); evacuation alternates ACT / DVE. A burst of
dummy matmuls during the prologue keeps the PE HAM at the warm 2.4 GHz clock.

Sharding: data-parallel over batch dim B=8 across the 8 NeuronCores.
"""

import os
import numpy as np

B, N, M, C = 8, 128, 32, 600
HUMAN_ID = 49
P = N * N
N_WARMUP_MM = 48
ST_CHUNK = 32

_CACHE = {}


def _build_nc():
    import concourse.bass as bass
    import concourse.bacc as bacc
    import concourse.tile as tile_mod
    from concourse import mybir
    from contextlib import ExitStack

    f32 = mybir.dt.float32
    bf16 = mybir.dt.bfloat16
    Alu = mybir.AluOpType

    nc = bacc.Bacc("TRN2")
    # boxesT is boxes transposed on the host to [4, N] so the on-chip
    # broadcast load is one contiguous-stride DMA.
    t_boxes = nc.dram_tensor("boxesT", [4, N], f32, kind="ExternalInput")
    t_bh = nc.dram_tensor("bh", [M, 4], f32, kind="ExternalInput")
    t_bo = nc.dram_tensor("bo", [M, 4], f32, kind="ExternalInput")
    t_det = nc.dram_tensor("det", [N], f32, kind="ExternalInput")
    t_hoi = nc.dram_tensor("hoi", [M], f32, kind="ExternalInput")
    t_lab = nc.dram_tensor("labels", [P, C], f32, kind="ExternalOutput")
    lab_ap = t_lab.ap()

    with tile_mod.TileContext(nc) as tc:
        with ExitStack() as ctx:
            const = ctx.enter_context(tc.tile_pool(name="const", bufs=1))
            psum = ctx.enter_context(tc.tile_pool(name="psum", bufs=4, space="PSUM"))
            outp = ctx.enter_context(tc.tile_pool(name="outp", bufs=8))

            # ---- load inputs (tiny, all HWDGE) ----
            # bx[m, k, n] = boxesT[k, n] = boxes[n, k], broadcast over M parts
            bx = const.tile([M, 4, N], f32)
            nc.sync.dma_start(
                out=bx[:, :, :],
                in_=bass.AP(tensor=t_boxes, offset=0, ap=[[0, M], [1, 4 * N]]),
            )
            bht = const.tile([M, 4], f32)
            nc.sync.dma_start(out=bht[:, :], in_=t_bh.ap())
            bot = const.tile([M, 4], f32)
            nc.sync.dma_start(out=bot[:, :], in_=t_bo.ap())
            # det broadcast over M partitions: detb[m, n] = det[n]
            detb = const.tile([M, N], f32)
            nc.sync.dma_start(
                out=detb[:, :],
                in_=bass.AP(tensor=t_det, offset=0, ap=[[0, M], [1, N]]),
            )
            hoit = const.tile([M, 1], f32)
            nc.sync.dma_start(
                out=hoit[:, :],
                in_=bass.AP(tensor=t_hoi, offset=0, ap=[[1, M], [1, 1]]),
            )

            X1 = bx[:, 0, :]
            Y1 = bx[:, 1, :]
            X2 = bx[:, 2, :]
            Y2 = bx[:, 3, :]

            # ---- PE warm-up: dense dummy matmuls while the prologue runs ----
            # Keeps the PE HAM at the 8/8 (2.4 GHz) clock by the time the real
            # matmul stream starts; results are never read.
            warm = psum.tile([N, N], f32, tag="ps")
            for _ in range(N_WARMUP_MM):
                nc.tensor.matmul(
                    warm[:, :], bx[:, 0, :], bx[:, 1, :], start=True, stop=True
                )

            # ---- area of each det box, replicated on all M partitions ----
            dbx = const.tile([M, N], f32)
            nc.vector.tensor_sub(dbx, X2, X1)
            dby = const.tile([M, N], f32)
            nc.vector.tensor_sub(dby, Y2, Y1)
            areaB = const.tile([M, N], f32)
            nc.vector.tensor_mul(areaB, dbx, dby)

            # ---- binary IoU-match matrices vs boxes_h and boxes_o ----
            # match <=> iou >= 0.5 <=> 3*inter >= areaA + areaB  (areas > 0)
            def iou_bin(src, out_dtype, tag):
                Sx1, Sy1, Sx2, Sy2 = (src[:, k : k + 1] for k in range(4))
                lt = const.tile([M, N], f32, tag=f"lt{tag}")
                nc.vector.tensor_scalar_max(lt, X1, Sx1)
                rb = const.tile([M, N], f32, tag=f"rb{tag}")
                nc.vector.tensor_scalar_min(rb, X2, Sx2)
                w = const.tile([M, N], f32, tag=f"w{tag}")
                nc.vector.tensor_sub(w, rb, lt)
                nc.vector.tensor_scalar_max(w, w, 0.0)
                lty = const.tile([M, N], f32, tag=f"lty{tag}")
                nc.vector.tensor_scalar_max(lty, Y1, Sy1)
                rby = const.tile([M, N], f32, tag=f"rby{tag}")
                nc.vector.tensor_scalar_min(rby, Y2, Sy2)
                h = const.tile([M, N], f32, tag=f"h{tag}")
                nc.vector.tensor_sub(h, rby, lty)
                nc.vector.tensor_scalar_max(h, h, 0.0)
                inter = const.tile([M, N], f32, tag=f"inter{tag}")
                nc.vector.tensor_mul(inter, w, h)
                # inter := 3*inter - areaB
                nc.vector.tensor_scalar_mul(inter, inter, 3.0)
                nc.vector.tensor_sub(inter, inter, areaB)
                adx = const.tile([M, 1], f32, tag=f"adx{tag}")
                nc.vector.tensor_sub(adx, Sx2, Sx1)
                ady = const.tile([M, 1], f32, tag=f"ady{tag}")
                nc.vector.tensor_sub(ady, Sy2, Sy1)
                areaS = const.tile([M, 1], f32, tag=f"areaS{tag}")
                nc.vector.tensor_mul(areaS, adx, ady)
                binS = const.tile([M, N], out_dtype, tag=f"bin{tag}")
                nc.vector.tensor_scalar(
                    out=binS, in0=inter, scalar1=areaS, scalar2=None, op0=Alu.is_ge
                )
                return binS

            bin_h = iou_bin(bht, f32, "h")
            bin_o = iou_bin(bot, bf16, "o")

            # fold det_labels == HUMAN_ID into the human-side matrix
            vb = const.tile([M, N], f32)
            nc.vector.tensor_scalar(
                out=vb, in0=detb, scalar1=float(HUMAN_ID), scalar2=None,
                op0=Alu.is_equal,
            )
            hh_m = const.tile([M, N], bf16)
            nc.vector.tensor_mul(hh_m, bin_h, vb)

            # ---- onehot[m, c] = (hoi[m] == c) ----
            iota_c = const.tile([M, C], f32)
            nc.gpsimd.iota(
                iota_c[:, :], pattern=[[1, C]], base=0, channel_multiplier=0,
                allow_small_or_imprecise_dtypes=True,
            )
            onehot = const.tile([M, C], bf16)
            nc.vector.tensor_scalar(
                out=onehot, in0=iota_c, scalar1=hoit[:, 0:1], scalar2=None,
                op0=Alu.is_equal,
            )

            # ---- Sm[j, i] = 0 if i == j else 1  (per-partition scale cols) ----
            iota_d = const.tile([N, N], f32)
            nc.gpsimd.iota(
                iota_d[:, :], pattern=[[-1, N]], base=0, channel_multiplier=1,
                allow_small_or_imprecise_dtypes=True,
            )
            Sm = const.tile([N, N], f32)
            nc.vector.tensor_scalar(
                out=Sm, in0=iota_d, scalar1=0.0, scalar2=None, op0=Alu.not_equal
            )

            # ---- ST[m, i, j] = hh_m[m, i] * bin_o[m, j], chunked over i so
            #      the first matmuls start before the whole table is built ----
            n_chunks = N // ST_CHUNK
            STs = []
            for c4 in range(n_chunks):
                STc = const.tile([M, ST_CHUNK, N], bf16, tag=f"ST{c4}")
                nc.vector.tensor_tensor(
                    out=STc[:, :, :],
                    in0=bin_o[:, :].unsqueeze(1).broadcast_to([M, ST_CHUNK, N]),
                    in1=hh_m[:, c4 * ST_CHUNK : (c4 + 1) * ST_CHUNK]
                    .unsqueeze(2)
                    .broadcast_to([M, ST_CHUNK, N]),
                    op=Alu.mult,
                )
                STs.append(STc)

            # ---- main loop over human index i ----
            Sign = mybir.ActivationFunctionType.Sign
            for i in range(N):
                ps = psum.tile([N, C], f32, tag="ps")
                st_i = STs[i // ST_CHUNK][:, i % ST_CHUNK, :]
                nc.tensor.matmul(
                    ps[:, 0:512], st_i, onehot[:, 0:512], start=True, stop=True
                )
                nc.tensor.matmul(
                    ps[:, 512:C], st_i, onehot[:, 512:C], start=True, stop=True
                )
                ob = outp.tile([N, C], f32)
                if i % 3 == 0:
                    # Sign(psum * Sm[:, i]): psum in {0..32}; row j==i scaled to 0
                    nc.scalar.activation(
                        out=ob[:, :], in_=ps[:, :], func=Sign, bias=0.0,
                        scale=Sm[:, i : i + 1],
                    )
                else:
                    nc.vector.tensor_scalar(
                        out=ob[:, :], in0=ps[:, :], scalar1=0.5,
                        scalar2=Sm[:, i : i + 1], op0=Alu.is_ge, op1=Alu.mult,
                    )
                dma_eng = nc.sync if i % 2 == 0 else nc.scalar
                dma_eng.dma_start(
                    out=lab_ap[i * N : (i + 1) * N, :], in_=ob[:, :]
                )
    nc.compile()
    return nc


def _get_nc():
    if "nc" not in _CACHE:
        _CACHE["nc"] = _build_nc()
    return _CACHE["nc"]


def kernel(boxes, boxes_h, boxes_o, det_labels, hoi):
    from concourse import bass_utils

    boxes = np.asarray(boxes, dtype=np.float32)
    boxes_h = np.ascontiguousarray(np.asarray(boxes_h), dtype=np.float32)
    boxes_o = np.ascontiguousarray(np.asarray(boxes_o), dtype=np.float32)
    det_f = np.asarray(det_labels).astype(np.float32)
    hoi_f = np.asarray(hoi).astype(np.float32)

    nc = _get_nc()
    in_maps = [
        {
            "boxesT": np.ascontiguousarray(boxes[b].T),
            "bh": boxes_h[b],
            "bo": boxes_o[b],
            "det": np.ascontiguousarray(det_f[b]),
            "hoi": np.ascontiguousarray(hoi_f[b]),
        }
        for b in range(B)
    ]
    trace = bool(int(os.environ.get("KERNEL_TRACE", "0")))
    res = bass_utils.run_bass_kernel_spmd(
        nc, in_maps, core_ids=list(range(B)), trace=trace
    )
    _CACHE["last_results"] = res

    labels = np.stack([res.results[b]["labels"] for b in range(B)], axis=0)
    hi, oi = np.meshgrid(np.arange(N), np.arange(N), indexing="ij")
    paired_idx = np.stack([hi.ravel(), oi.ravel()], axis=-1).astype(np.int32)
    return paired_idx, labels


# revision 10
# speedup vs baseline: 1.2925x; 1.1395x over previous
"""Trainium2 Bass kernel for nn_InteractionHead (HOI pair label assignment).

Reference semantics (per image b):
    P = N*N candidate (human, object) index pairs (i, j).
    valid[p=(i,j)] = (det_labels[i] == 49) and (i != j)
    iou[p, m] = min(IoU(boxes[i], boxes_h[m]), IoU(boxes[j], boxes_o[m]))
    labels[p, c] = valid[p] * min(1, sum_m (iou[p,m] >= 0.5) * (hoi[m] == c))

Key identity: IoU(boxes[i], boxes_h[m]) only depends on (i, m), so the whole
problem reduces to two [M, N] binary match matrices per image:
    Hh[m, i] = IoU(boxes[i], boxes_h[m]) >= 0.5   (masked by det_labels[i]==49)
    Ho[m, j] = IoU(boxes[j], boxes_o[m]) >= 0.5
    labels[(i,j), c] = clip(sum_m Hh[m,i] * Ho[m,j] * onehot[m,c]) * (i != j)

Per core (1 image): precompute ST[m, i, j] = Hh[m,i]*Ho[m,j] (bf16, exact 0/1,
built in 4 chunks so matmuls can start early), then for each i: matmul
ST[:, i, :].T @ onehot -> psum[128 j, 600 c], evacuate with a clip-to-{0,1} op
whose per-partition scale operand zeroes row j == i, DMA the [128, 600] f32
tile to rows [i*128, (i+1)*128) of the output. Output stores alternate between
the two HWDGE rings (SP / ACT); evacuation alternates ACT / DVE. A burst of
dummy matmuls during the prologue keeps the PE HAM at the warm 2.4 GHz clock.

Sharding: data-parallel over batch dim B=8 across the 8 NeuronCores.
"""

import os
import numpy as np

B, N, M, C = 8, 128, 32, 600
HUMAN_ID = 49
P = N * N
N_WARMUP_MM = 48
ST_CHUNK = 32

_CACHE = {}


def _build_nc():
    import concourse.bass as bass
    import concourse.bacc as bacc
    import concourse.tile as tile_mod
    from concourse import mybir
    from contextlib import ExitStack

    f32 = mybir.dt.float32
    bf16 = mybir.dt.bfloat16
    Alu = mybir.AluOpType

    nc = bacc.Bacc("TRN2")
    # boxesT is boxes transposed on the host to [4, N] so the on-chip
    # broadcast load is one contiguous-stride DMA.
    t_boxes = nc.dram_tensor("boxesT", [4, N], f32, kind="ExternalInput")
    t_bh = nc.dram_tensor("bh", [M, 4], f32, kind="ExternalInput")
    t_bo = nc.dram_tensor("bo", [M, 4], f32, kind="ExternalInput")
    t_det = nc.dram_tensor("det", [N], f32, kind="ExternalInput")
    t_hoi = nc.dram_tensor("hoi", [M], f32, kind="ExternalInput")
    t_lab = nc.dram_tensor("labels", [P, C], f32, kind="ExternalOutput")
    lab_ap = t_lab.ap()

    with tile_mod.TileContext(nc) as tc:
        with ExitStack() as ctx:
            const = ctx.enter_context(tc.tile_pool(name="const", bufs=1))
            psum = ctx.enter_context(tc.tile_pool(name="psum", bufs=4, space="PSUM"))
            outp = ctx.enter_context(tc.tile_pool(name="outp", bufs=8))

            # ---- load inputs (tiny, all HWDGE) ----
            # bx[m, k, n] = boxesT[k, n] = boxes[n, k], broadcast over M parts
            bx = const.tile([M, 4, N], f32)
            nc.sync.dma_start(
                out=bx[:, :, :],
                in_=bass.AP(tensor=t_boxes, offset=0, ap=[[0, M], [1, 4 * N]]),
            )
            bht = const.tile([M, 4], f32)
            nc.sync.dma_start(out=bht[:, :], in_=t_bh.ap())
            bot = const.tile([M, 4], f32)
            nc.sync.dma_start(out=bot[:, :], in_=t_bo.ap())
            # det broadcast over M partitions: detb[m, n] = det[n]
            detb = const.tile([M, N], f32)
            nc.sync.dma_start(
                out=detb[:, :],
                in_=bass.AP(tensor=t_det, offset=0, ap=[[0, M], [1, N]]),
            )
            hoit = const.tile([M, 1], f32)
            nc.sync.dma_start(
                out=hoit[:, :],
                in_=bass.AP(tensor=t_hoi, offset=0, ap=[[1, M], [1, 1]]),
            )

            X1 = bx[:, 0, :]
            Y1 = bx[:, 1, :]
            X2 = bx[:, 2, :]
            Y2 = bx[:, 3, :]

            # ---- area of each det box, replicated on all M partitions ----
            dbx = const.tile([M, N], f32)
            nc.vector.tensor_sub(dbx, X2, X1)
            dby = const.tile([M, N], f32)
            nc.vector.tensor_sub(dby, Y2, Y1)
            areaB = const.tile([M, N], f32)
            nc.vector.tensor_mul(areaB, dbx, dby)

            # ---- binary IoU-match matrices vs boxes_h and boxes_o ----
            # match <=> iou >= 0.5 <=> 3*inter >= areaA + areaB  (areas > 0)
            def iou_bin(src, out_dtype, tag):
                Sx1, Sy1, Sx2, Sy2 = (src[:, k : k + 1] for k in range(4))
                lt = const.tile([M, N], f32, tag=f"lt{tag}")
                nc.vector.tensor_scalar_max(lt, X1, Sx1)
                rb = const.tile([M, N], f32, tag=f"rb{tag}")
                nc.vector.tensor_scalar_min(rb, X2, Sx2)
                w = const.tile([M, N], f32, tag=f"w{tag}")
                nc.vector.tensor_sub(w, rb, lt)
                nc.vector.tensor_scalar_max(w, w, 0.0)
                lty = const.tile([M, N], f32, tag=f"lty{tag}")
                nc.vector.tensor_scalar_max(lty, Y1, Sy1)
                rby = const.tile([M, N], f32, tag=f"rby{tag}")
                nc.vector.tensor_scalar_min(rby, Y2, Sy2)
                h = const.tile([M, N], f32, tag=f"h{tag}")
                nc.vector.tensor_sub(h, rby, lty)
                nc.vector.tensor_scalar_max(h, h, 0.0)
                inter = const.tile([M, N], f32, tag=f"inter{tag}")
                nc.vector.tensor_mul(inter, w, h)
                # inter := 3*inter - areaB
                nc.vector.tensor_scalar_mul(inter, inter, 3.0)
                nc.vector.tensor_sub(inter, inter, areaB)
                adx = const.tile([M, 1], f32, tag=f"adx{tag}")
                nc.vector.tensor_sub(adx, Sx2, Sx1)
                ady = const.tile([M, 1], f32, tag=f"ady{tag}")
                nc.vector.tensor_sub(ady, Sy2, Sy1)
                areaS = const.tile([M, 1], f32, tag=f"areaS{tag}")
                nc.vector.tensor_mul(areaS, adx, ady)
                binS = const.tile([M, N], out_dtype, tag=f"bin{tag}")
                nc.vector.tensor_scalar(
                    out=binS, in0=inter, scalar1=areaS, scalar2=None, op0=Alu.is_ge
                )
                return binS

            bin_h = iou_bin(bht, f32, "h")
            bin_o = iou_bin(bot, bf16, "o")

            # fold det_labels == HUMAN_ID into the human-side matrix
            vb = const.tile([M, N], f32)
            nc.vector.tensor_scalar(
                out=vb, in0=detb, scalar1=float(HUMAN_ID), scalar2=None,
                op0=Alu.is_equal,
            )
            hh_m = const.tile([M, N], bf16)
            nc.vector.tensor_mul(hh_m, bin_h, vb)

            # ---- onehot[m, c] = (hoi[m] == c) ----
            iota_c = const.tile([M, C], f32)
            nc.gpsimd.iota(
                iota_c[:, :], pattern=[[1, C]], base=0, channel_multiplier=0,
                allow_small_or_imprecise_dtypes=True,
            )
            onehot = const.tile([M, C], bf16)
            nc.vector.tensor_scalar(
                out=onehot, in0=iota_c, scalar1=hoit[:, 0:1], scalar2=None,
                op0=Alu.is_equal,
            )

            # ---- Sm[j, i] = 0 if i == j else 1  (per-partition scale cols) ----
            iota_d = const.tile([N, N], f32)
            nc.gpsimd.iota(
                iota_d[:, :], pattern=[[-1, N]], base=0, channel_multiplier=1,
                allow_small_or_imprecise_dtypes=True,
            )
            Sm = const.tile([N, N], f32)
            nc.vector.tensor_scalar(
                out=Sm, in0=iota_d, scalar1=0.0, scalar2=None, op0=Alu.not_equal
            )

            # ---- ST[m, i, j] = hh_m[m, i] * bin_o[m, j], chunked over i so
            #      the first matmuls start before the whole table is built.
            #      Small leading chunks gate the loop start; big ones follow. ----
            chunk_sizes = [16, 16, 32, 32, 32]
            STs = []  # list of (i0, size, tile)
            i0 = 0
            for ci, csz in enumerate(chunk_sizes):
                STc = const.tile([M, csz, N], bf16, tag=f"ST{ci}")
                nc.vector.tensor_tensor(
                    out=STc[:, :, :],
                    in0=bin_o[:, :].unsqueeze(1).broadcast_to([M, csz, N]),
                    in1=hh_m[:, i0 : i0 + csz]
                    .unsqueeze(2)
                    .broadcast_to([M, csz, N]),
                    op=Alu.mult,
                )
                STs.append((i0, csz, STc))
                i0 += csz

            def st_slice(i):
                for c0, csz, STc in STs:
                    if c0 <= i < c0 + csz:
                        return STc[:, i - c0, :]
                raise AssertionError

            # ---- main loop over human index i ----
            Sign = mybir.ActivationFunctionType.Sign
            for i in range(N):
                ps = psum.tile([N, C], f32, tag="ps")
                st_i = st_slice(i)
                nc.tensor.matmul(
                    ps[:, 0:512], st_i, onehot[:, 0:512], start=True, stop=True
                )
                nc.tensor.matmul(
                    ps[:, 512:C], st_i, onehot[:, 512:C], start=True, stop=True
                )
                ob = outp.tile([N, C], f32)
                if i % 3 == 0:
                    # Sign(psum * Sm[:, i]): psum in {0..32}; row j==i scaled to 0
                    nc.scalar.activation(
                        out=ob[:, :], in_=ps[:, :], func=Sign, bias=0.0,
                        scale=Sm[:, i : i + 1],
                    )
                else:
                    nc.vector.tensor_scalar(
                        out=ob[:, :], in0=ps[:, :], scalar1=0.5,
                        scalar2=Sm[:, i : i + 1], op0=Alu.is_ge, op1=Alu.mult,
                    )
                dma_eng = nc.sync if i % 2 == 0 else nc.scalar
                dma_eng.dma_start(
                    out=lab_ap[i * N : (i + 1) * N, :], in_=ob[:, :]
                )
    nc.compile()
    return nc


def _get_nc():
    if "nc" not in _CACHE:
        _CACHE["nc"] = _build_nc()
    return _CACHE["nc"]


def kernel(boxes, boxes_h, boxes_o, det_labels, hoi):
    from concourse import bass_utils

    boxes = np.asarray(boxes, dtype=np.float32)
    boxes_h = np.ascontiguousarray(np.asarray(boxes_h), dtype=np.float32)
    boxes_o = np.ascontiguousarray(np.asarray(boxes_o), dtype=np.float32)
    det_f = np.asarray(det_labels).astype(np.float32)
    hoi_f = np.asarray(hoi).astype(np.float32)

    nc = _get_nc()
    in_maps = [
        {
            "boxesT": np.ascontiguousarray(boxes[b].T),
            "bh": boxes_h[b],
            "bo": boxes_o[b],
            "det": np.ascontiguousarray(det_f[b]),
            "hoi": np.ascontiguousarray(hoi_f[b]),
        }
        for b in range(B)
    ]
    trace = bool(int(os.environ.get("KERNEL_TRACE", "0")))
    res = bass_utils.run_bass_kernel_spmd(
        nc, in_maps, core_ids=list(range(B)), trace=trace
    )
    _CACHE["last_results"] = res

    labels = np.stack([res.results[b]["labels"] for b in range(B)], axis=0)
    hi, oi = np.meshgrid(np.arange(N), np.arange(N), indexing="ij")
    paired_idx = np.stack([hi.ravel(), oi.ravel()], axis=-1).astype(np.int32)
    return paired_idx, labels
